# revision 2
# baseline (speedup 1.0000x reference)
"""Fused attention kernel for Trainium2 (Bass/Tile), 8-core data-parallel.

Problem (nn_AttentionModel): B=8, L=2048, V=1024, D=512
    q = x @ Wq.T ; k = x @ Wk.T ; v = x @ Wv.T          (per batch element)
    out = softmax(q @ k.T / sqrt(D)) @ v

Sharding: data-parallel over batch — core b gets x[b] plus replicated
weights, computes its full attention on-chip, no collectives.

Per-core dataflow (all matmul operands bf16, fp32 PSUM accumulation):
  1. DMA x [L,V] f32 -> cast bf16 -> PE-transpose 128x128 blocks -> xT [V,L]
     (contractions need v on the partition dim). Same for Wq/Wk/Wv -> WT [V,D].
  2. Projections on TensorE:
        qT[d,l], kT[d,l]  (lhsT=WT tile, rhs=xT)   — transposed layout
        v[l,d]            (lhsT=xT tile, rhs=WvT)  — natural layout,
     with an extra all-ones column appended to v (rowsum trick below).
  3. Per 512-wide q block: scores.T tile [k,q] = kT.T @ qT on TensorE,
     exp(scale*s) on ScalarE straight out of PSUM into bf16 P.T tiles.
     No max-subtraction: |scores/sqrt(D)| < ~3 here, exp cannot overflow.
  4. AV on TensorE: lhsT=P.T tile, rhs=v' -> psum [q,513] where column 512
     accumulates sum_k P[q,k] (the softmax denominator) for free, already
     in per-partition layout. reciprocal + tensor_scalar_mul -> out rows.
"""

import math
import sys

sys.path.insert(0, "/opt/trn_rl_repo")

import numpy as np

import concourse.bacc as bacc
import concourse.bass as bass
import concourse.tile as tile
from concourse import mybir
from concourse.bass_utils import run_bass_kernel_spmd
from concourse.masks import make_identity

B, L, V, D = 8, 2048, 1024, 512
P = 128
LT, VT, DT = L // P, V // P, D // P      # 16, 8, 4
QM = 512                                  # q columns processed per block
NQM = L // QM                             # 4
SCALE = 1.0 / math.sqrt(D)

F32 = mybir.dt.float32
BF16 = mybir.dt.bfloat16

N_CORES = 8


def _build_attention(tc: tile.TileContext, out, x, wq, wk, wv, ctx):
    nc = tc.nc

    sb = ctx.enter_context(tc.tile_pool(name="sb", bufs=1))
    stage = ctx.enter_context(tc.tile_pool(name="stage", bufs=3))
    psum = ctx.enter_context(tc.tile_pool(name="psum", bufs=4, space="PSUM"))
    psum_av = ctx.enter_context(tc.tile_pool(name="psum_av", bufs=2, space="PSUM"))
    ptp = ctx.enter_context(tc.tile_pool(name="ptp", bufs=2))
    outp = ctx.enter_context(tc.tile_pool(name="outp", bufs=4))

    identity = sb.tile([P, P], BF16)
    make_identity(nc, identity)

    # Persistent on-chip tensors (layouts: [partition, tile_idx, free])
    xT = sb.tile([P, VT, L], BF16)    # xT[p,vt,l]  = x[l, vt*P+p]
    wqT = sb.tile([P, VT, D], BF16)   # wqT[p,vt,d] = Wq[d, vt*P+p]
    wkT = sb.tile([P, VT, D], BF16)
    wvT = sb.tile([P, VT, D], BF16)
    qT = sb.tile([P, DT, L], BF16)    # qT[p,m,l] = q[l, m*P+p]
    kT = sb.tile([P, DT, L], BF16)
    vO = sb.tile([P, LT, D + 1], BF16)  # vO[p,lt,d] = v[lt*P+p, d]; [:,:,D] = 1

    # ---- weights: load, cast, transpose (PE) ----
    for w_dram, wT in ((wk, wkT), (wq, wqT), (wv, wvT)):
        for di in range(DT):
            w_nat = stage.tile([P, V], F32, tag="stage_f32")
            nc.sync.dma_start(out=w_nat, in_=w_dram[di * P:(di + 1) * P, :])
            w_bf = stage.tile([P, V], BF16, tag="stage_bf16")
            nc.vector.tensor_copy(out=w_bf, in_=w_nat)
            for vt in range(VT):
                pt = psum.tile([P, P], BF16, tag="mm")
                nc.tensor.transpose(pt, w_bf[:, vt * P:(vt + 1) * P], identity)
                nc.vector.tensor_copy(out=wT[:, vt, di * P:(di + 1) * P], in_=pt)

    # ---- x: load, cast, transpose (PE) ----
    for lt in range(LT):
        x_nat = stage.tile([P, V], F32, tag="stage_f32")
        nc.sync.dma_start(out=x_nat, in_=x[lt * P:(lt + 1) * P, :])
        x_bf = stage.tile([P, V], BF16, tag="stage_bf16")
        nc.vector.tensor_copy(out=x_bf, in_=x_nat)
        for vt in range(VT):
            pt = psum.tile([P, P], BF16, tag="mm")
            nc.tensor.transpose(pt, x_bf[:, vt * P:(vt + 1) * P], identity)
            nc.vector.tensor_copy(out=xT[:, vt, lt * P:(lt + 1) * P], in_=pt)

    # ---- projections: kT, qT in [d, l] layout ----
    for wT, oT in ((wkT, kT), (wqT, qT)):
        for m in range(DT):
            for n in range(NQM):
                ps = psum.tile([P, QM], F32, tag="mm")
                for vt in range(VT):
                    nc.tensor.matmul(
                        ps,
                        lhsT=wT[:, vt, m * P:(m + 1) * P],
                        rhs=xT[:, vt, n * QM:(n + 1) * QM],
                        start=(vt == 0),
                        stop=(vt == VT - 1),
                    )
                nc.vector.tensor_copy(out=oT[:, m, n * QM:(n + 1) * QM], in_=ps)

    # ---- projection: v in natural [l, d] layout + ones column ----
    for lt in range(LT):
        ps = psum.tile([P, D], F32, tag="mm")
        for vt in range(VT):
            nc.tensor.matmul(
                ps,
                lhsT=xT[:, vt, lt * P:(lt + 1) * P],
                rhs=wvT[:, vt, :],
                start=(vt == 0),
                stop=(vt == VT - 1),
            )
        nc.vector.tensor_copy(out=vO[:, lt, :D], in_=ps)
        nc.vector.memset(vO[:, lt, D:D + 1], 1.0)

    # ---- attention, one 512-wide q block at a time ----
    for qm in range(NQM):
        PT = ptp.tile([P, LT, QM], BF16, tag="PT")  # P.T[k, q-block]
        for kt in range(LT):
            ps = psum.tile([P, QM], F32, tag="mm")
            for m in range(DT):
                nc.tensor.matmul(
                    ps,
                    lhsT=kT[:, m, kt * P:(kt + 1) * P],
                    rhs=qT[:, m, qm * QM:(qm + 1) * QM],
                    start=(m == 0),
                    stop=(m == DT - 1),
                )
            nc.scalar.activation(
                out=PT[:, kt, :], in_=ps,
                func=mybir.ActivationFunctionType.Exp, scale=SCALE,
            )
        for qs in range(QM // P):
            pa = psum_av.tile([P, D], F32, tag="av")
            prs = psum_av.tile([P, 1], F32, tag="rs")
            for kt in range(LT):
                pT_tile = PT[:, kt, qs * P:(qs + 1) * P]
                nc.tensor.matmul(
                    pa, lhsT=pT_tile, rhs=vO[:, kt, :D],
                    start=(kt == 0), stop=(kt == LT - 1),
                )
                nc.tensor.matmul(
                    prs, lhsT=pT_tile, rhs=vO[:, kt, D:D + 1],
                    start=(kt == 0), stop=(kt == LT - 1),
                )
            rs = outp.tile([P, 1], F32, tag="rs_sb")
            nc.vector.reciprocal(rs, prs)
            ot = outp.tile([P, D], F32, tag="ot")
            nc.vector.tensor_scalar_mul(ot, pa, rs)
            lq = qm * QM + qs * P
            nc.sync.dma_start(out=out[lq:lq + P, :], in_=ot)


_NC_CACHE = None


def _get_nc():
    global _NC_CACHE
    if _NC_CACHE is not None:
        return _NC_CACHE
    from contextlib import ExitStack

    nc = bacc.Bacc("TRN2", target_bir_lowering=False, debug=False,
                   num_devices=N_CORES)
    x = nc.declare_dram_parameter("x", [L, V], F32, isOutput=False)
    wq = nc.declare_dram_parameter("Wq", [D, V], F32, isOutput=False)
    wk = nc.declare_dram_parameter("Wk", [D, V], F32, isOutput=False)
    wv = nc.declare_dram_parameter("Wv", [D, V], F32, isOutput=False)
    out = nc.declare_dram_parameter("out", [L, D], F32, isOutput=True)
    with tile.TileContext(nc) as tc:
        with ExitStack() as ctx:
            _build_attention(tc, out.ap(), x.ap(), wq.ap(), wk.ap(), wv.ap(), ctx)
    nc.compile()
    _NC_CACHE = nc
    return nc


def _run(x, Wq, Wk, Wv, **spmd_kwargs):
    nc = _get_nc()
    x = np.ascontiguousarray(np.asarray(x, dtype=np.float32))
    Wq = np.ascontiguousarray(np.asarray(Wq, dtype=np.float32))
    Wk = np.ascontiguousarray(np.asarray(Wk, dtype=np.float32))
    Wv = np.ascontiguousarray(np.asarray(Wv, dtype=np.float32))
    in_maps = [
        {"x": np.ascontiguousarray(x[b]), "Wq": Wq, "Wk": Wk, "Wv": Wv}
        for b in range(N_CORES)
    ]
    res = run_bass_kernel_spmd(nc, in_maps, core_ids=list(range(N_CORES)),
                               **spmd_kwargs)
    out = np.stack([res.results[b]["out"] for b in range(N_CORES)], axis=0)
    return out, res


def kernel(x, Wq, Wk, Wv):
    out, _ = _run(x, Wq, Wk, Wv)
    return out
